# revision 9
# baseline (speedup 1.0000x reference)
"""Trainium2 Bass kernel for nn_BAMM (pooled self-attention block + residual).

Reference computation (per batch sample, B=8 sharded 1/core over 8 cores):
  x  = avg_pool4(input)          [512, 32, 32] -> flat [512, 1024]
  y  = avg_pool4(c2)
  q  = Wq @ x + bq               [128, 1024]
  k  = Wk @ y + bk               [128, 1024]
  v  = Wv @ y + bv               [512, 1024]
  E  = (q^T k) / sqrt(128)       [1024, 1024]
  A  = softmax(E, axis=-1)
  o  = v @ A^T                   [512, 1024]
  out = upsample4(o) + c2        [512, 128, 128]

Device-side layout strategy (one sample per core):
  - Pooling as sum-pool on DVE (tensor_reduce XY over a 5D AP view); the
    1/16 is folded into the host-preprocessed weights.
  - All matmuls in float32r (full PE rate at N=512, fp32 storage).
  - Attention computed transposed: eT[m,n] = sum_ck k[ck,m] q[ck,n] so that
    the value bmm contracts over m on the partition dim with no transposes.
  - softmax denominators via ones-matmul (column sums of exp, broadcast
    across partitions); bv folded into vT so out = (vT' @ expT) * recip.
  - exp without max-subtraction: energies are O(0.1) by construction.
"""

import sys
import types

import numpy as np

import bass_rust

import concourse.bass as bass
import concourse.tile as tile
from concourse import mybir
from concourse.bass_utils import run_bass_kernel_spmd
from concourse.vector_clock import ScopedClock


class _TileContextPatched(tile.TileContext):
    """Work around a walrus sync-wait-count limit: the stock kernel-tail
    InstDrain carries every outstanding sem wait; this walrus build rejects
    more than one sync wait on a Drain. Spread the surplus across nofuse NOPs.
    """

    def _drain_and_barrier(self, tick_clock, wait_clock):
        nc = self.nc
        drain_inst = nc.sync.drain()
        wait_clock.add_sem_waits(
            drain_inst.ins, ScopedClock({None: tick_clock.global_clock})
        )
        si = drain_inst.ins.sync_info
        if si is not None and si.on_wait and len(si.on_wait) > 1:
            waits = list(si.on_wait)
            si.on_wait = waits[:1]
            for i in range(1, len(waits)):
                nop = nc.sync.nop(nofuse=True)
                nop.ins.sync_info = bass_rust.SyncInfo(
                    on_wait=waits[i:i + 1], on_update=[]
                )

        nc.all_engine_barrier()
        assert self.sems is not None
        popped = nc._tile_sem_poison_stack.pop()
        assert popped is self._sem_poison
        nc.clear_and_free_semaphores(list(self.sems.allocated().values()))
        nc.all_engine_barrier()

F32 = mybir.dt.float32
BF16 = mybir.dt.bfloat16

_MW_COUNTER = [0]


def _split_multi_waits(nc, max_waits=1):
    """This walrus build encodes at most one sync wait per instruction.
    Hoist surplus waits onto same-engine NoOps inserted just before the
    over-subscribed instruction (engine programs execute in order, so the
    NoOps block the engine until every wait is satisfied)."""
    for f in nc.m.functions:
        for bb in f.blocks:
            new_list = []
            for ins in bb.instructions:
                si = ins.sync_info
                if si is not None and si.on_wait and len(si.on_wait) > max_waits:
                    waits = list(si.on_wait)
                    extras, keep = waits[:-max_waits], waits[-max_waits:]
                    for w in extras:
                        _MW_COUNTER[0] += 1
                        nop = bass_rust.InstNoOp(
                            name=f"I-mw{_MW_COUNTER[0]}", engine=ins.engine
                        )
                        nop.sync_info = bass_rust.SyncInfo(
                            on_wait=[w], on_update=[]
                        )
                        new_list.append(nop)
                    si.on_wait = keep
                new_list.append(ins)
            bb.instructions[:] = new_list

P = 128          # partitions
C = 512          # channels
CT = C // P      # 4 channel tiles
H = 128          # input spatial
DS = 4           # pool factor
HP = H // DS     # 32 pooled spatial
N2 = HP * HP     # 1024 pooled positions
NH = 2           # halves of N2 (512 each, one PSUM bank)
MT = N2 // P     # 8 m-tiles
HCG = 4          # h-chunks per channel tile (32 input rows each)
CHUNK = 32 * H   # 4096 elements per partition per chunk
CK = 128         # q/k channels


def _install_ntff_shim():
    """Register the axon NTFF profile hook if the image's antenv lacks it."""
    try:
        import antenv.axon_hooks  # noqa: F401
        return
    except ImportError:
        pass
    try:
        from trn_agent_boot.trn_boot import _ntff_profile_via_ctypes
        hook = _ntff_profile_via_ctypes("/opt/axon/libaxon_pjrt.so")
        m = types.ModuleType("antenv.axon_hooks")
        m.get_axon_ntff_profile_hook = lambda: hook
        sys.modules["antenv.axon_hooks"] = m
    except Exception:
        pass


def build_nc(split_waits=True):
    nc = bass.Bass()

    inp = nc.declare_dram_parameter("inp", [C, H, H], F32, isOutput=False)
    c2s = nc.declare_dram_parameter("c2s", [C, H, H], F32, isOutput=False)
    # host-preprocessed weights: wq = Wq.T * scale/16, wk = Wk.T/16, wv = Wv.T/16
    wq = nc.declare_dram_parameter("wq", [C, CK], BF16, isOutput=False)
    wk = nc.declare_dram_parameter("wk", [C, CK], BF16, isOutput=False)
    wv = nc.declare_dram_parameter("wv", [C, C], BF16, isOutput=False)
    bq = nc.declare_dram_parameter("bq", [CK, 1], F32, isOutput=False)  # * scale
    bk = nc.declare_dram_parameter("bk", [CK, 1], F32, isOutput=False)
    bv = nc.declare_dram_parameter("bv", [C], F32, isOutput=False)
    out = nc.declare_dram_parameter("outp", [C, H, H], F32, isOutput=True)

    with _TileContextPatched(nc) as tc:
        _emit(nc, tc, inp, c2s, wq, wk, wv, bq, bk, bv, out)
    if split_waits:
        _split_multi_waits(nc)
    return nc


def _emit(nc, tc, inp, c2s, wq, wk, wv, bq, bk, bv, out):
    from contextlib import ExitStack

    ctx = ExitStack()
    with ctx:
        const = ctx.enter_context(tc.tile_pool(name="const", bufs=1))
        feat = ctx.enter_context(tc.tile_pool(name="feat", bufs=1))
        stream = ctx.enter_context(tc.tile_pool(name="stream", bufs=3))
        psum = ctx.enter_context(tc.tile_pool(name="psum", bufs=2, space="PSUM"))

        # ---- constants ----
        wq_sb = [const.tile([P, CK], BF16, tag=f"wq{i}", name=f"wq{i}") for i in range(CT)]
        wk_sb = [const.tile([P, CK], BF16, tag=f"wk{i}", name=f"wk{i}") for i in range(CT)]
        wv_sb = [const.tile([P, C], BF16, tag=f"wv{i}", name=f"wv{i}") for i in range(CT)]
        for i in range(CT):
            nc.sync.dma_start(out=wq_sb[i][:], in_=wq[i * P:(i + 1) * P, :])
            nc.sync.dma_start(out=wk_sb[i][:], in_=wk[i * P:(i + 1) * P, :])
            nc.sync.dma_start(out=wv_sb[i][:], in_=wv[i * P:(i + 1) * P, :])
        bq_sb = const.tile([P, 1], F32, tag="bq")
        bk_sb = const.tile([P, 1], F32, tag="bk")
        nc.sync.dma_start(out=bq_sb[:], in_=bq[:])
        nc.sync.dma_start(out=bk_sb[:], in_=bk[:])
        bv_sb = const.tile([P, C], F32, tag="bv")
        nc.gpsimd.dma_start(out=bv_sb[:], in_=bv[:].partition_broadcast(P))
        ones_sb = const.tile([P, P], BF16, tag="ones")
        nc.vector.memset(ones_sb[:], 1.0)

        # ---- persistent feature tiles ----
        xf = [feat.tile([P, N2], BF16, tag=f"xf{i}", name=f"xf{i}") for i in range(CT)]
        yf = [feat.tile([P, N2], BF16, tag=f"yf{i}", name=f"yf{i}") for i in range(CT)]
        q_sb = feat.tile([P, N2], BF16, tag="q")
        k_sb = feat.tile([P, N2], BF16, tag="k")
        vt_sb = [feat.tile([P, C], BF16, tag=f"vt{i}", name=f"vt{i}") for i in range(MT)]
        et_sb = [feat.tile([P, N2], BF16, tag=f"et{i}", name=f"et{i}") for i in range(MT)]
        recip = feat.tile([P, N2], F32, tag="recip")
        onrm = [feat.tile([P, N2], F32, tag=f"onrm{i}", name=f"onrm{i}") for i in range(CT)]

        def pool_chunk(dst_ap, src_dram, ct, hcg):
            t = stream.tile([P, CHUNK], F32, tag="stream")
            nc.sync.dma_start(
                out=t[:],
                in_=src_dram[ct * P:(ct + 1) * P, hcg * 32:(hcg + 1) * 32, :],
            )
            # [p, (h' i w' j)] -> [p, h', w', i, j], reduce (i, j)
            v5 = t[:].rearrange(
                "p (a b c d) -> p a c b d", a=8, b=DS, c=HP, d=DS
            )
            with nc.allow_low_precision(reason="pool sums written bf16 for PE"):
                nc.vector.reduce_sum(
                    out=dst_ap[:, hcg * 256:(hcg + 1) * 256],
                    in_=v5,
                    axis=mybir.AxisListType.XY,
                )

        # ---- Phase A: stream + pool ----
        for ct in range(CT):
            for hcg in range(HCG):
                pool_chunk(xf[ct], inp, ct, hcg)
        for ct in range(CT):
            for hcg in range(HCG):
                pool_chunk(yf[ct], c2s, ct, hcg)

        # ---- Phase B ----
        def nhs(ap, nh):
            return ap[:, nh * 512:(nh + 1) * 512]

        # q, k
        for nh in range(NH):
            qp = psum.tile([P, 512], F32, tag="acc")
            for ct in range(CT):
                nc.tensor.matmul(
                    qp[:], wq_sb[ct][:],
                    nhs(xf[ct], nh),
                    start=(ct == 0), stop=(ct == CT - 1),
                )
            nc.vector.tensor_scalar_add(nhs(q_sb, nh), qp[:], bq_sb[:])
        for nh in range(NH):
            kp = psum.tile([P, 512], F32, tag="acc")
            for ct in range(CT):
                nc.tensor.matmul(
                    kp[:], wk_sb[ct][:],
                    nhs(yf[ct], nh),
                    start=(ct == 0), stop=(ct == CT - 1),
                )
            nc.vector.tensor_scalar_add(nhs(k_sb, nh), kp[:], bk_sb[:])

        # vT (bv folded in)
        for mt in range(MT):
            vp = psum.tile([P, 512], F32, tag="acc")
            for ct in range(CT):
                nc.tensor.matmul(
                    vp[:],
                    yf[ct][:, mt * P:(mt + 1) * P],
                    wv_sb[ct][:],
                    start=(ct == 0), stop=(ct == CT - 1),
                )
            nc.vector.tensor_add(vt_sb[mt][:], vp[:], bv_sb[:])

        # energyT + exp + column-sum accumulation
        sp = [psum.tile([P, 512], F32, tag=f"sp{nh}", name=f"sp{nh}", bufs=1) for nh in range(NH)]
        for mt in range(MT):
            for nh in range(NH):
                ep = psum.tile([P, 512], F32, tag="ep")
                nc.tensor.matmul(
                    ep[:],
                    k_sb[:, mt * P:(mt + 1) * P],
                    nhs(q_sb, nh),
                    start=True, stop=True,
                )
                nc.scalar.activation(
                    out=nhs(et_sb[mt], nh), in_=ep[:],
                    func=mybir.ActivationFunctionType.Exp,
                )
            for nh in range(NH):
                nc.tensor.matmul(
                    sp[nh][:], ones_sb[:],
                    nhs(et_sb[mt], nh),
                    start=(mt == 0), stop=(mt == MT - 1),
                )
        for nh in range(NH):
            nc.vector.reciprocal(nhs(recip, nh), sp[nh][:])

        # out bmm + normalize
        for ct in range(CT):
            for nh in range(NH):
                op = psum.tile([P, 512], F32, tag="op")
                for mt in range(MT):
                    nc.tensor.matmul(
                        op[:],
                        vt_sb[mt][:, ct * P:(ct + 1) * P],
                        nhs(et_sb[mt], nh),
                        start=(mt == 0), stop=(mt == MT - 1),
                    )
                nc.vector.tensor_mul(nhs(onrm[ct], nh), op[:], nhs(recip, nh))

        # ---- Phase C: residual + upsample ----
        for ct in range(CT):
            for hcg in range(HCG):
                t = stream.tile([P, CHUNK], F32, tag="stream")
                nc.sync.dma_start(
                    out=t[:],
                    in_=c2s[ct * P:(ct + 1) * P, hcg * 32:(hcg + 1) * 32, :],
                )
                tv = t[:].rearrange(
                    "p (a b c d) -> p a b c d", a=8, b=DS, c=HP, d=DS
                )
                ons = onrm[ct][:, hcg * 256:(hcg + 1) * 256].rearrange(
                    "p (a c) -> p a c", a=8
                )
                for i in range(DS):
                    for j in range(DS):
                        nc.vector.tensor_add(
                            tv[:, :, i, :, j], tv[:, :, i, :, j], ons
                        )
                nc.scalar.dma_start(
                    out=out[ct * P:(ct + 1) * P, hcg * 32:(hcg + 1) * 32, :],
                    in_=t[:],
                )


_NC_CACHE = None


def _get_nc():
    global _NC_CACHE
    if _NC_CACHE is None:
        _install_ntff_shim()
        _NC_CACHE = build_nc()
    return _NC_CACHE


def prep_weights(Wq, bq, Wk, bk, Wv, bv):
    scale = np.float32(1.0 / np.sqrt(np.float32(CK)))
    sixteenth = np.float32(1.0 / 16.0)
    import ml_dtypes
    bf16 = ml_dtypes.bfloat16
    return {
        "wq": np.ascontiguousarray((Wq.T * (scale * sixteenth)).astype(bf16)),
        "wk": np.ascontiguousarray((Wk.T * sixteenth).astype(bf16)),
        "wv": np.ascontiguousarray((Wv.T * sixteenth).astype(bf16)),
        "bq": np.ascontiguousarray((bq * scale).reshape(CK, 1), dtype=np.float32),
        "bk": np.ascontiguousarray(bk.reshape(CK, 1), dtype=np.float32),
        "bv": np.ascontiguousarray(bv, dtype=np.float32),
    }


def kernel(input, c2, Wq, bq, Wk, bk, Wv, bv, _trace=False):
    input = np.asarray(input, dtype=np.float32)
    c2 = np.asarray(c2, dtype=np.float32)
    w = prep_weights(
        np.asarray(Wq, np.float32), np.asarray(bq, np.float32),
        np.asarray(Wk, np.float32), np.asarray(bk, np.float32),
        np.asarray(Wv, np.float32), np.asarray(bv, np.float32),
    )
    B = input.shape[0]
    nc = _get_nc()
    in_maps = [
        {"inp": np.ascontiguousarray(input[i]),
         "c2s": np.ascontiguousarray(c2[i]), **w}
        for i in range(B)
    ]
    res = run_bass_kernel_spmd(nc, in_maps, list(range(B)), trace=_trace)
    outp = np.stack([res.results[i]["outp"] for i in range(B)])
    if _trace:
        kernel._last_result = res
    return outp


# revision 11
# speedup vs baseline: 1.0766x; 1.0766x over previous
"""Trainium2 Bass kernel for nn_BAMM (pooled self-attention block + residual).

Reference computation (per batch sample, B=8 sharded 1/core over 8 cores):
  x  = avg_pool4(input)          [512, 32, 32] -> flat [512, 1024]
  y  = avg_pool4(c2)
  q  = Wq @ x + bq               [128, 1024]
  k  = Wk @ y + bk               [128, 1024]
  v  = Wv @ y + bv               [512, 1024]
  E  = (q^T k) / sqrt(128)       [1024, 1024]
  A  = softmax(E, axis=-1)
  o  = v @ A^T                   [512, 1024]
  out = upsample4(o) + c2        [512, 128, 128]

Device-side layout strategy (one sample per core):
  - Pooling as sum-pool on DVE (tensor_reduce XY over a 5D AP view); the
    1/16 is folded into the host-preprocessed weights.
  - All matmuls in float32r (full PE rate at N=512, fp32 storage).
  - Attention computed transposed: eT[m,n] = sum_ck k[ck,m] q[ck,n] so that
    the value bmm contracts over m on the partition dim with no transposes.
  - softmax denominators via ones-matmul (column sums of exp, broadcast
    across partitions); bv folded into vT so out = (vT' @ expT) * recip.
  - exp without max-subtraction: energies are O(0.1) by construction.
"""

import sys
import types

import numpy as np

import bass_rust

import concourse.bass as bass
import concourse.tile as tile
from concourse import mybir
from concourse.bass_utils import run_bass_kernel_spmd
from concourse.vector_clock import ScopedClock


class _TileContextPatched(tile.TileContext):
    """Work around a walrus sync-wait-count limit: the stock kernel-tail
    InstDrain carries every outstanding sem wait; this walrus build rejects
    more than one sync wait on a Drain. Spread the surplus across nofuse NOPs.
    """

    def _drain_and_barrier(self, tick_clock, wait_clock):
        nc = self.nc
        drain_inst = nc.sync.drain()
        wait_clock.add_sem_waits(
            drain_inst.ins, ScopedClock({None: tick_clock.global_clock})
        )
        si = drain_inst.ins.sync_info
        if si is not None and si.on_wait and len(si.on_wait) > 1:
            waits = list(si.on_wait)
            si.on_wait = waits[:1]
            for i in range(1, len(waits)):
                nop = nc.sync.nop(nofuse=True)
                nop.ins.sync_info = bass_rust.SyncInfo(
                    on_wait=waits[i:i + 1], on_update=[]
                )

        nc.all_engine_barrier()
        assert self.sems is not None
        popped = nc._tile_sem_poison_stack.pop()
        assert popped is self._sem_poison
        nc.clear_and_free_semaphores(list(self.sems.allocated().values()))
        nc.all_engine_barrier()

F32 = mybir.dt.float32
BF16 = mybir.dt.bfloat16

_MW_COUNTER = [0]


def _split_multi_waits(nc, max_waits=1):
    """This walrus build encodes at most one sync wait per instruction.
    Hoist surplus waits onto same-engine NoOps inserted just before the
    over-subscribed instruction (engine programs execute in order, so the
    NoOps block the engine until every wait is satisfied)."""
    for f in nc.m.functions:
        for bb in f.blocks:
            new_list = []
            for ins in bb.instructions:
                si = ins.sync_info
                if si is not None and si.on_wait and len(si.on_wait) > max_waits:
                    waits = list(si.on_wait)
                    extras, keep = waits[:-max_waits], waits[-max_waits:]
                    for w in extras:
                        _MW_COUNTER[0] += 1
                        nop = bass_rust.InstNoOp(
                            name=f"I-mw{_MW_COUNTER[0]}", engine=ins.engine
                        )
                        nop.sync_info = bass_rust.SyncInfo(
                            on_wait=[w], on_update=[]
                        )
                        new_list.append(nop)
                    si.on_wait = keep
                new_list.append(ins)
            bb.instructions[:] = new_list

P = 128          # partitions
C = 512          # channels
CT = C // P      # 4 channel tiles
H = 128          # input spatial
DS = 4           # pool factor
HP = H // DS     # 32 pooled spatial
N2 = HP * HP     # 1024 pooled positions
NH = 2           # halves of N2 (512 each, one PSUM bank)
MT = N2 // P     # 8 m-tiles
HCG = 2          # h-chunks per channel tile (64 input rows each)
CHUNK = 64 * H   # 8192 elements per partition per chunk (4 MB tiles)
HPC = 16         # pooled h-rows per chunk
CK = 128         # q/k channels


def _install_ntff_shim():
    """Register the axon NTFF profile hook if the image's antenv lacks it."""
    try:
        import antenv.axon_hooks  # noqa: F401
        return
    except ImportError:
        pass
    try:
        from trn_agent_boot.trn_boot import _ntff_profile_via_ctypes
        hook = _ntff_profile_via_ctypes("/opt/axon/libaxon_pjrt.so")
        m = types.ModuleType("antenv.axon_hooks")
        m.get_axon_ntff_profile_hook = lambda: hook
        sys.modules["antenv.axon_hooks"] = m
    except Exception:
        pass


def build_nc(split_waits=True):
    nc = bass.Bass()

    inp = nc.declare_dram_parameter("inp", [C, H, H], F32, isOutput=False)
    c2s = nc.declare_dram_parameter("c2s", [C, H, H], F32, isOutput=False)
    # host-preprocessed weights: wq = Wq.T * scale/16, wk = Wk.T/16, wv = Wv.T/16
    wq = nc.declare_dram_parameter("wq", [C, CK], BF16, isOutput=False)
    wk = nc.declare_dram_parameter("wk", [C, CK], BF16, isOutput=False)
    wv = nc.declare_dram_parameter("wv", [C, C], BF16, isOutput=False)
    bq = nc.declare_dram_parameter("bq", [CK, 1], F32, isOutput=False)  # * scale
    bk = nc.declare_dram_parameter("bk", [CK, 1], F32, isOutput=False)
    bv = nc.declare_dram_parameter("bv", [C], F32, isOutput=False)
    out = nc.declare_dram_parameter("outp", [C, H, H], F32, isOutput=True)

    with _TileContextPatched(nc) as tc:
        _emit(nc, tc, inp, c2s, wq, wk, wv, bq, bk, bv, out)
    if split_waits:
        _split_multi_waits(nc)
    return nc


def _emit(nc, tc, inp, c2s, wq, wk, wv, bq, bk, bv, out):
    from contextlib import ExitStack

    ctx = ExitStack()
    with ctx:
        const = ctx.enter_context(tc.tile_pool(name="const", bufs=1))
        feat = ctx.enter_context(tc.tile_pool(name="feat", bufs=1))
        stream = ctx.enter_context(tc.tile_pool(name="stream", bufs=3))
        psum = ctx.enter_context(tc.tile_pool(name="psum", bufs=2, space="PSUM"))

        # ---- constants ----
        wq_sb = [const.tile([P, CK], BF16, tag=f"wq{i}", name=f"wq{i}") for i in range(CT)]
        wk_sb = [const.tile([P, CK], BF16, tag=f"wk{i}", name=f"wk{i}") for i in range(CT)]
        wv_sb = [const.tile([P, C], BF16, tag=f"wv{i}", name=f"wv{i}") for i in range(CT)]
        for i in range(CT):
            nc.sync.dma_start(out=wq_sb[i][:], in_=wq[i * P:(i + 1) * P, :])
            nc.sync.dma_start(out=wk_sb[i][:], in_=wk[i * P:(i + 1) * P, :])
            nc.sync.dma_start(out=wv_sb[i][:], in_=wv[i * P:(i + 1) * P, :])
        bq_sb = const.tile([P, 1], F32, tag="bq")
        bk_sb = const.tile([P, 1], F32, tag="bk")
        nc.sync.dma_start(out=bq_sb[:], in_=bq[:])
        nc.sync.dma_start(out=bk_sb[:], in_=bk[:])
        bv_sb = const.tile([P, C], F32, tag="bv")
        nc.gpsimd.dma_start(out=bv_sb[:], in_=bv[:].partition_broadcast(P))
        ones_sb = const.tile([P, P], BF16, tag="ones")
        nc.vector.memset(ones_sb[:], 1.0)

        # ---- persistent feature tiles ----
        xf = [feat.tile([P, N2], BF16, tag=f"xf{i}", name=f"xf{i}") for i in range(CT)]
        yf = [feat.tile([P, N2], BF16, tag=f"yf{i}", name=f"yf{i}") for i in range(CT)]
        q_sb = feat.tile([P, N2], BF16, tag="q")
        k_sb = feat.tile([P, N2], BF16, tag="k")
        vt_sb = [feat.tile([P, C], BF16, tag=f"vt{i}", name=f"vt{i}") for i in range(MT)]
        et_sb = [feat.tile([P, N2], BF16, tag=f"et{i}", name=f"et{i}") for i in range(MT)]
        recip = feat.tile([P, N2], F32, tag="recip")
        onrm = [feat.tile([P, N2], F32, tag=f"onrm{i}", name=f"onrm{i}") for i in range(CT)]

        def pool_chunk(dst_ap, src_dram, ct, hcg):
            t = stream.tile([P, CHUNK], F32, tag="stream")
            nc.sync.dma_start(
                out=t[:],
                in_=src_dram[ct * P:(ct + 1) * P, hcg * 64:(hcg + 1) * 64, :],
            )
            # [p, (h' i w' j)] -> [p, h', w', i, j], reduce (i, j)
            v5 = t[:].rearrange(
                "p (a b c d) -> p a c b d", a=HPC, b=DS, c=HP, d=DS
            )
            with nc.allow_low_precision(reason="pool sums written bf16 for PE"):
                nc.vector.reduce_sum(
                    out=dst_ap[:, hcg * HPC * HP:(hcg + 1) * HPC * HP],
                    in_=v5,
                    axis=mybir.AxisListType.XY,
                )

        # ---- Phase A: stream + pool ----
        for ct in range(CT):
            for hcg in range(HCG):
                pool_chunk(xf[ct], inp, ct, hcg)
        for ct in range(CT):
            for hcg in range(HCG):
                pool_chunk(yf[ct], c2s, ct, hcg)

        # ---- Phase B ----
        def nhs(ap, nh):
            return ap[:, nh * 512:(nh + 1) * 512]

        # q, k
        for nh in range(NH):
            qp = psum.tile([P, 512], F32, tag="acc")
            for ct in range(CT):
                nc.tensor.matmul(
                    qp[:], wq_sb[ct][:],
                    nhs(xf[ct], nh),
                    start=(ct == 0), stop=(ct == CT - 1),
                )
            nc.vector.tensor_scalar_add(nhs(q_sb, nh), qp[:], bq_sb[:])
        for nh in range(NH):
            kp = psum.tile([P, 512], F32, tag="acc")
            for ct in range(CT):
                nc.tensor.matmul(
                    kp[:], wk_sb[ct][:],
                    nhs(yf[ct], nh),
                    start=(ct == 0), stop=(ct == CT - 1),
                )
            nc.vector.tensor_scalar_add(nhs(k_sb, nh), kp[:], bk_sb[:])

        # vT (bv folded in)
        for mt in range(MT):
            vp = psum.tile([P, 512], F32, tag="acc")
            for ct in range(CT):
                nc.tensor.matmul(
                    vp[:],
                    yf[ct][:, mt * P:(mt + 1) * P],
                    wv_sb[ct][:],
                    start=(ct == 0), stop=(ct == CT - 1),
                )
            nc.vector.tensor_add(vt_sb[mt][:], vp[:], bv_sb[:])

        # energyT + exp + column-sum accumulation
        sp = [psum.tile([P, 512], F32, tag=f"sp{nh}", name=f"sp{nh}", bufs=1) for nh in range(NH)]
        for mt in range(MT):
            for nh in range(NH):
                ep = psum.tile([P, 512], F32, tag="ep")
                nc.tensor.matmul(
                    ep[:],
                    k_sb[:, mt * P:(mt + 1) * P],
                    nhs(q_sb, nh),
                    start=True, stop=True,
                )
                nc.scalar.activation(
                    out=nhs(et_sb[mt], nh), in_=ep[:],
                    func=mybir.ActivationFunctionType.Exp,
                )
            for nh in range(NH):
                nc.tensor.matmul(
                    sp[nh][:], ones_sb[:],
                    nhs(et_sb[mt], nh),
                    start=(mt == 0), stop=(mt == MT - 1),
                )
        for nh in range(NH):
            nc.vector.reciprocal(nhs(recip, nh), sp[nh][:])

        # out bmm + normalize
        for ct in range(CT):
            for nh in range(NH):
                op = psum.tile([P, 512], F32, tag="op")
                for mt in range(MT):
                    nc.tensor.matmul(
                        op[:],
                        vt_sb[mt][:, ct * P:(ct + 1) * P],
                        nhs(et_sb[mt], nh),
                        start=(mt == 0), stop=(mt == MT - 1),
                    )
                nc.vector.tensor_mul(nhs(onrm[ct], nh), op[:], nhs(recip, nh))

        # ---- Phase C: residual + upsample ----
        for ct in range(CT):
            for hcg in range(HCG):
                t = stream.tile([P, CHUNK], F32, tag="stream")
                nc.sync.dma_start(
                    out=t[:],
                    in_=c2s[ct * P:(ct + 1) * P, hcg * 64:(hcg + 1) * 64, :],
                )
                tv = t[:].rearrange(
                    "p (a b c d) -> p a b c d", a=HPC, b=DS, c=HP, d=DS
                )
                ons = (
                    onrm[ct][:, hcg * HPC * HP:(hcg + 1) * HPC * HP]
                    .rearrange("p (a c) -> p a c", a=HPC)
                    .unsqueeze(3)
                    .broadcast_to([P, HPC, HP, DS])
                )
                for i in range(DS):
                    nc.vector.tensor_add(
                        tv[:, :, i, :, :], tv[:, :, i, :, :], ons
                    )
                nc.scalar.dma_start(
                    out=out[ct * P:(ct + 1) * P, hcg * 64:(hcg + 1) * 64, :],
                    in_=t[:],
                )


_NC_CACHE = None


def _get_nc():
    global _NC_CACHE
    if _NC_CACHE is None:
        _install_ntff_shim()
        _NC_CACHE = build_nc()
    return _NC_CACHE


def prep_weights(Wq, bq, Wk, bk, Wv, bv):
    scale = np.float32(1.0 / np.sqrt(np.float32(CK)))
    sixteenth = np.float32(1.0 / 16.0)
    import ml_dtypes
    bf16 = ml_dtypes.bfloat16
    return {
        "wq": np.ascontiguousarray((Wq.T * (scale * sixteenth)).astype(bf16)),
        "wk": np.ascontiguousarray((Wk.T * sixteenth).astype(bf16)),
        "wv": np.ascontiguousarray((Wv.T * sixteenth).astype(bf16)),
        "bq": np.ascontiguousarray((bq * scale).reshape(CK, 1), dtype=np.float32),
        "bk": np.ascontiguousarray(bk.reshape(CK, 1), dtype=np.float32),
        "bv": np.ascontiguousarray(bv, dtype=np.float32),
    }


def kernel(input, c2, Wq, bq, Wk, bk, Wv, bv, _trace=False):
    input = np.asarray(input, dtype=np.float32)
    c2 = np.asarray(c2, dtype=np.float32)
    w = prep_weights(
        np.asarray(Wq, np.float32), np.asarray(bq, np.float32),
        np.asarray(Wk, np.float32), np.asarray(bk, np.float32),
        np.asarray(Wv, np.float32), np.asarray(bv, np.float32),
    )
    B = input.shape[0]
    nc = _get_nc()
    in_maps = [
        {"inp": np.ascontiguousarray(input[i]),
         "c2s": np.ascontiguousarray(c2[i]), **w}
        for i in range(B)
    ]
    res = run_bass_kernel_spmd(nc, in_maps, list(range(B)), trace=_trace)
    outp = np.stack([res.results[i]["outp"] for i in range(B)])
    if _trace:
        kernel._last_result = res
    return outp


# revision 12
# speedup vs baseline: 1.3010x; 1.2084x over previous
"""Trainium2 Bass kernel for nn_BAMM (pooled self-attention block + residual).

Reference computation (per batch sample, B=8 sharded 1/core over 8 cores):
  x  = avg_pool4(input)          [512, 32, 32] -> flat [512, 1024]
  y  = avg_pool4(c2)
  q  = Wq @ x + bq               [128, 1024]
  k  = Wk @ y + bk               [128, 1024]
  v  = Wv @ y + bv               [512, 1024]
  E  = (q^T k) / sqrt(128)       [1024, 1024]
  A  = softmax(E, axis=-1)
  o  = v @ A^T                   [512, 1024]
  out = upsample4(o) + c2        [512, 128, 128]

Device-side layout strategy (one sample per core):
  - Pooling as sum-pool on DVE (tensor_reduce XY over a 5D AP view); the
    1/16 is folded into the host-preprocessed weights.
  - All matmuls in float32r (full PE rate at N=512, fp32 storage).
  - Attention computed transposed: eT[m,n] = sum_ck k[ck,m] q[ck,n] so that
    the value bmm contracts over m on the partition dim with no transposes.
  - softmax denominators via ones-matmul (column sums of exp, broadcast
    across partitions); bv folded into vT so out = (vT' @ expT) * recip.
  - exp without max-subtraction: energies are O(0.1) by construction.
"""

import sys
import types

import numpy as np

import bass_rust

import concourse.bass as bass
import concourse.tile as tile
from concourse import mybir
from concourse.bass_utils import run_bass_kernel_spmd
from concourse.vector_clock import ScopedClock


class _TileContextPatched(tile.TileContext):
    """Work around a walrus sync-wait-count limit: the stock kernel-tail
    InstDrain carries every outstanding sem wait; this walrus build rejects
    more than one sync wait on a Drain. Spread the surplus across nofuse NOPs.
    """

    def _drain_and_barrier(self, tick_clock, wait_clock):
        nc = self.nc
        drain_inst = nc.sync.drain()
        wait_clock.add_sem_waits(
            drain_inst.ins, ScopedClock({None: tick_clock.global_clock})
        )
        si = drain_inst.ins.sync_info
        if si is not None and si.on_wait and len(si.on_wait) > 1:
            waits = list(si.on_wait)
            si.on_wait = waits[:1]
            for i in range(1, len(waits)):
                nop = nc.sync.nop(nofuse=True)
                nop.ins.sync_info = bass_rust.SyncInfo(
                    on_wait=waits[i:i + 1], on_update=[]
                )

        nc.all_engine_barrier()
        assert self.sems is not None
        popped = nc._tile_sem_poison_stack.pop()
        assert popped is self._sem_poison
        nc.clear_and_free_semaphores(list(self.sems.allocated().values()))
        nc.all_engine_barrier()

F32 = mybir.dt.float32
BF16 = mybir.dt.bfloat16

_MW_COUNTER = [0]


def _split_multi_waits(nc, max_waits=1):
    """This walrus build encodes at most one sync wait per instruction.
    Hoist surplus waits onto same-engine NoOps inserted just before the
    over-subscribed instruction (engine programs execute in order, so the
    NoOps block the engine until every wait is satisfied)."""
    for f in nc.m.functions:
        for bb in f.blocks:
            new_list = []
            for ins in bb.instructions:
                si = ins.sync_info
                if si is not None and si.on_wait and len(si.on_wait) > max_waits:
                    waits = list(si.on_wait)
                    extras, keep = waits[:-max_waits], waits[-max_waits:]
                    for w in extras:
                        _MW_COUNTER[0] += 1
                        nop = bass_rust.InstNoOp(
                            name=f"I-mw{_MW_COUNTER[0]}", engine=ins.engine
                        )
                        nop.sync_info = bass_rust.SyncInfo(
                            on_wait=[w], on_update=[]
                        )
                        new_list.append(nop)
                    si.on_wait = keep
                new_list.append(ins)
            bb.instructions[:] = new_list

P = 128          # partitions
C = 512          # channels
CT = C // P      # 4 channel tiles
H = 128          # input spatial
DS = 4           # pool factor
HP = H // DS     # 32 pooled spatial
N2 = HP * HP     # 1024 pooled positions
NH = 2           # halves of N2 (512 each, one PSUM bank)
MT = N2 // P     # 8 m-tiles
HCG = 4          # h-chunks per channel tile (32 input rows each)
CHUNK = 32 * H   # 4096 elements per partition per chunk (2 MB tiles)
HPC = 8          # pooled h-rows per chunk
CK = 128         # q/k channels


def _install_ntff_shim():
    """Register the axon NTFF profile hook if the image's antenv lacks it."""
    try:
        import antenv.axon_hooks  # noqa: F401
        return
    except ImportError:
        pass
    try:
        from trn_agent_boot.trn_boot import _ntff_profile_via_ctypes
        hook = _ntff_profile_via_ctypes("/opt/axon/libaxon_pjrt.so")
        m = types.ModuleType("antenv.axon_hooks")
        m.get_axon_ntff_profile_hook = lambda: hook
        sys.modules["antenv.axon_hooks"] = m
    except Exception:
        pass


def build_nc(split_waits=True):
    nc = bass.Bass()

    inp = nc.declare_dram_parameter("inp", [C, H, H], F32, isOutput=False)
    c2s = nc.declare_dram_parameter("c2s", [C, H, H], F32, isOutput=False)
    # host-preprocessed weights: wq = Wq.T * scale/16, wk = Wk.T/16, wv = Wv.T/16
    wq = nc.declare_dram_parameter("wq", [C, CK], BF16, isOutput=False)
    wk = nc.declare_dram_parameter("wk", [C, CK], BF16, isOutput=False)
    wv = nc.declare_dram_parameter("wv", [C, C], BF16, isOutput=False)
    bq = nc.declare_dram_parameter("bq", [CK, 1], F32, isOutput=False)  # * scale
    bk = nc.declare_dram_parameter("bk", [CK, 1], F32, isOutput=False)
    bv = nc.declare_dram_parameter("bv", [C], F32, isOutput=False)
    out = nc.declare_dram_parameter("outp", [C, H, H], F32, isOutput=True)

    with _TileContextPatched(nc) as tc:
        _emit(nc, tc, inp, c2s, wq, wk, wv, bq, bk, bv, out)
    if split_waits:
        _split_multi_waits(nc)
    return nc


def _emit(nc, tc, inp, c2s, wq, wk, wv, bq, bk, bv, out):
    from contextlib import ExitStack

    ctx = ExitStack()
    with ctx:
        const = ctx.enter_context(tc.tile_pool(name="const", bufs=1))
        feat = ctx.enter_context(tc.tile_pool(name="feat", bufs=1))
        stream = ctx.enter_context(tc.tile_pool(name="stream", bufs=6))
        psum = ctx.enter_context(tc.tile_pool(name="psum", bufs=2, space="PSUM"))

        # ---- constants ----
        wq_sb = [const.tile([P, CK], BF16, tag=f"wq{i}", name=f"wq{i}") for i in range(CT)]
        wk_sb = [const.tile([P, CK], BF16, tag=f"wk{i}", name=f"wk{i}") for i in range(CT)]
        wv_sb = [const.tile([P, C], BF16, tag=f"wv{i}", name=f"wv{i}") for i in range(CT)]
        for i in range(CT):
            nc.sync.dma_start(out=wq_sb[i][:], in_=wq[i * P:(i + 1) * P, :])
            nc.sync.dma_start(out=wk_sb[i][:], in_=wk[i * P:(i + 1) * P, :])
            nc.sync.dma_start(out=wv_sb[i][:], in_=wv[i * P:(i + 1) * P, :])
        bq_sb = const.tile([P, 1], F32, tag="bq")
        bk_sb = const.tile([P, 1], F32, tag="bk")
        nc.sync.dma_start(out=bq_sb[:], in_=bq[:])
        nc.sync.dma_start(out=bk_sb[:], in_=bk[:])
        bv_sb = const.tile([P, C], F32, tag="bv")
        nc.gpsimd.dma_start(out=bv_sb[:], in_=bv[:].partition_broadcast(P))
        ones_sb = const.tile([P, P], BF16, tag="ones")
        nc.vector.memset(ones_sb[:], 1.0)

        # ---- persistent feature tiles ----
        xf = [feat.tile([P, N2], BF16, tag=f"xf{i}", name=f"xf{i}") for i in range(CT)]
        yf = [feat.tile([P, N2], BF16, tag=f"yf{i}", name=f"yf{i}") for i in range(CT)]
        q_sb = feat.tile([P, N2], BF16, tag="q")
        k_sb = feat.tile([P, N2], BF16, tag="k")
        vt_sb = [feat.tile([P, C], BF16, tag=f"vt{i}", name=f"vt{i}") for i in range(MT)]
        et_sb = [feat.tile([P, N2], BF16, tag=f"et{i}", name=f"et{i}") for i in range(MT)]
        recip = feat.tile([P, N2], F32, tag="recip")
        onrm = [feat.tile([P, N2], F32, tag=f"onrm{i}", name=f"onrm{i}") for i in range(CT)]

        def pool_chunk(dst_ap, src_dram, ct, hcg):
            t = stream.tile([P, CHUNK], F32, tag="stream")
            nc.sync.dma_start(
                out=t[:],
                in_=src_dram[ct * P:(ct + 1) * P, hcg * 32:(hcg + 1) * 32, :],
            )
            # [p, (h' i w' j)] -> [p, h', w', i, j], reduce (i, j)
            v5 = t[:].rearrange(
                "p (a b c d) -> p a c b d", a=HPC, b=DS, c=HP, d=DS
            )
            with nc.allow_low_precision(reason="pool sums written bf16 for PE"):
                nc.vector.reduce_sum(
                    out=dst_ap[:, hcg * HPC * HP:(hcg + 1) * HPC * HP],
                    in_=v5,
                    axis=mybir.AxisListType.XY,
                )

        # ---- Phase A: stream + pool (c2 first so k/vT hide under inp) ----
        for ct in range(CT):
            for hcg in range(HCG):
                pool_chunk(yf[ct], c2s, ct, hcg)
        for ct in range(CT):
            for hcg in range(HCG):
                pool_chunk(xf[ct], inp, ct, hcg)

        # ---- Phase B ----
        def nhs(ap, nh):
            return ap[:, nh * 512:(nh + 1) * 512]

        # k, vT depend only on yf -> run under the inp stream
        for nh in range(NH):
            kp = psum.tile([P, 512], F32, tag="acc")
            for ct in range(CT):
                nc.tensor.matmul(
                    kp[:], wk_sb[ct][:],
                    nhs(yf[ct], nh),
                    start=(ct == 0), stop=(ct == CT - 1),
                )
            nc.vector.tensor_scalar_add(nhs(k_sb, nh), kp[:], bk_sb[:])

        # vT (bv folded in)
        for mt in range(MT):
            vp = psum.tile([P, 512], F32, tag="acc")
            for ct in range(CT):
                nc.tensor.matmul(
                    vp[:],
                    yf[ct][:, mt * P:(mt + 1) * P],
                    wv_sb[ct][:],
                    start=(ct == 0), stop=(ct == CT - 1),
                )
            nc.vector.tensor_add(vt_sb[mt][:], vp[:], bv_sb[:])

        # q (gated on the full inp stream)
        for nh in range(NH):
            qp = psum.tile([P, 512], F32, tag="acc")
            for ct in range(CT):
                nc.tensor.matmul(
                    qp[:], wq_sb[ct][:],
                    nhs(xf[ct], nh),
                    start=(ct == 0), stop=(ct == CT - 1),
                )
            nc.vector.tensor_scalar_add(nhs(q_sb, nh), qp[:], bq_sb[:])

        # PE warm-up: ~8 matmuls gated on the last inp tile so they run at
        # the stream tail and lift HAM to 8/8 before the energy chain.
        wu = psum.tile([P, 512], F32, tag="op", name="wu")
        for r in range(8):
            nc.tensor.matmul(
                wu[:], wq_sb[r % CT][:], nhs(xf[CT - 1], r % NH),
                start=True, stop=True,
            )
        wu_guard = feat.tile([P, 1], F32, tag="wug")
        nc.vector.tensor_copy(wu_guard[:], wu[:, 0:1])

        # energyT + exp + column-sum accumulation
        sp = [psum.tile([P, 512], F32, tag=f"sp{nh}", name=f"sp{nh}", bufs=1) for nh in range(NH)]
        for mt in range(MT):
            for nh in range(NH):
                ep = psum.tile([P, 512], F32, tag="ep")
                nc.tensor.matmul(
                    ep[:],
                    k_sb[:, mt * P:(mt + 1) * P],
                    nhs(q_sb, nh),
                    start=True, stop=True,
                )
                nc.scalar.activation(
                    out=nhs(et_sb[mt], nh), in_=ep[:],
                    func=mybir.ActivationFunctionType.Exp,
                )
            for nh in range(NH):
                nc.tensor.matmul(
                    sp[nh][:], ones_sb[:],
                    nhs(et_sb[mt], nh),
                    start=(mt == 0), stop=(mt == MT - 1),
                )
        for nh in range(NH):
            nc.vector.reciprocal(nhs(recip, nh), sp[nh][:])

        # out bmm + normalize (nh innermost so each vt weight-load serves 2 mms)
        for ct in range(CT):
            ops = [psum.tile([P, 512], F32, tag="op", name=f"op{ct}_{nh}")
                   for nh in range(NH)]
            for mt in range(MT):
                for nh in range(NH):
                    nc.tensor.matmul(
                        ops[nh][:],
                        vt_sb[mt][:, ct * P:(ct + 1) * P],
                        nhs(et_sb[mt], nh),
                        start=(mt == 0), stop=(mt == MT - 1),
                    )
            for nh in range(NH):
                nc.vector.tensor_mul(nhs(onrm[ct], nh), ops[nh][:], nhs(recip, nh))

        # ---- Phase C: residual + upsample ----
        for ct in range(CT):
            for hcg in range(HCG):
                t = stream.tile([P, CHUNK], F32, tag="stream")
                nc.sync.dma_start(
                    out=t[:],
                    in_=c2s[ct * P:(ct + 1) * P, hcg * 32:(hcg + 1) * 32, :],
                )
                tv = t[:].rearrange(
                    "p (a b c d) -> p a b c d", a=HPC, b=DS, c=HP, d=DS
                )
                ons = (
                    onrm[ct][:, hcg * HPC * HP:(hcg + 1) * HPC * HP]
                    .rearrange("p (a c) -> p a c", a=HPC)
                    .unsqueeze(3)
                    .broadcast_to([P, HPC, HP, DS])
                )
                for i in range(DS):
                    nc.vector.tensor_add(
                        tv[:, :, i, :, :], tv[:, :, i, :, :], ons
                    )
                nc.scalar.dma_start(
                    out=out[ct * P:(ct + 1) * P, hcg * 32:(hcg + 1) * 32, :],
                    in_=t[:],
                )


_NC_CACHE = None


def _get_nc():
    global _NC_CACHE
    if _NC_CACHE is None:
        _install_ntff_shim()
        _NC_CACHE = build_nc()
    return _NC_CACHE


def prep_weights(Wq, bq, Wk, bk, Wv, bv):
    scale = np.float32(1.0 / np.sqrt(np.float32(CK)))
    sixteenth = np.float32(1.0 / 16.0)
    import ml_dtypes
    bf16 = ml_dtypes.bfloat16
    return {
        "wq": np.ascontiguousarray((Wq.T * (scale * sixteenth)).astype(bf16)),
        "wk": np.ascontiguousarray((Wk.T * sixteenth).astype(bf16)),
        "wv": np.ascontiguousarray((Wv.T * sixteenth).astype(bf16)),
        "bq": np.ascontiguousarray((bq * scale).reshape(CK, 1), dtype=np.float32),
        "bk": np.ascontiguousarray(bk.reshape(CK, 1), dtype=np.float32),
        "bv": np.ascontiguousarray(bv, dtype=np.float32),
    }


def kernel(input, c2, Wq, bq, Wk, bk, Wv, bv, _trace=False):
    input = np.asarray(input, dtype=np.float32)
    c2 = np.asarray(c2, dtype=np.float32)
    w = prep_weights(
        np.asarray(Wq, np.float32), np.asarray(bq, np.float32),
        np.asarray(Wk, np.float32), np.asarray(bk, np.float32),
        np.asarray(Wv, np.float32), np.asarray(bv, np.float32),
    )
    B = input.shape[0]
    nc = _get_nc()
    in_maps = [
        {"inp": np.ascontiguousarray(input[i]),
         "c2s": np.ascontiguousarray(c2[i]), **w}
        for i in range(B)
    ]
    res = run_bass_kernel_spmd(nc, in_maps, list(range(B)), trace=_trace)
    outp = np.stack([res.results[i]["outp"] for i in range(B)])
    if _trace:
        kernel._last_result = res
    return outp
